# revision 26
# baseline (speedup 1.0000x reference)
"""MoE router kernel for Trainium2 (8 NeuronCores, SPMD data-parallel).

Computes, for x [B,S,H] and gate_w [E,H]:
    logits = x @ gate_w.T           # [B,S,E]
    p = softmax(logits, -1)
    w, i = top_k(p, 2); w = w / w.sum(-1, keepdims=True)

Math used on-device: renormalized top-2 softmax weights collapse to
    w1 = sigmoid(l1 - l2), w2 = sigmoid(l2 - l1)
where l1 >= l2 are the top-2 logits, so the full softmax is never needed.

Sharding: tokens (B*S = 16384) split evenly across 8 cores; gate weights
replicated. Per core: 2048 tokens x 4096 hidden.

Per-core pipeline (groups of 512 tokens = 4 x-tiles of [128, 4096]):
  DMA x tile [128, 4096] fp32 -> PE fp32 transposes of 128x128 chunks
  into PSUM banks [128h, 512t] -> DVE/ACT copy to SBUF -> fp32r GEMM
  (w chunk [128h, 64e] stationary, xT [128h, 512t] moving, 1 cyc/row)
  accumulating logitsT [64e, 512t] in PSUM -> copy to SBUF -> 4 PE
  transposes back to [128t, 64e] -> DVE max/max_index (top-8 sorted) ->
  ACT sigmoid -> DMA out.

fp32r (reduced-precision fp32 matmul, ~12.7 effective mantissa bits
measured on HW) is used ONLY for the final contraction against the tiny
gate weights; x itself is transposed at full fp32. Measured index
mismatch stays well inside the 2e-2 gate. Set GEMM_MODE="fp16x3" for a
bit-near-fp32 3-pass split-fp16 GEMM instead (slower, ~20 mantissa
bits).
"""

import sys

sys.path.insert(0, "/opt/trn_rl_repo")

import numpy as np

import concourse.bass as bass
import concourse.mybir as mybir
import concourse.tile as tile
from concourse.bass_utils import run_bass_kernel_spmd
import orjson
import concourse.bass_utils as _bu
import concourse.bass2jax as _b2j

_orig_compile_bir = _bu.compile_bir_kernel


def _legalize_waits(bir_json: bytes) -> bytes:
    """This walrus build allows only ONE sync-wait per compute
    instruction; move excess waits onto a Drain inserted just before
    (Drain accepts one wait each)."""
    m = orjson.loads(bir_json)
    changed = False
    for fn in m["functions"]:
        for blk in fn["blocks"]:
            out = []
            for inst in blk["instructions"]:
                si = inst.get("sync_info")
                w = (si or {}).get("on_wait") or []
                if len(w) > 1:
                    for k, wk in enumerate(w[:-1]):
                        out.append({
                            "debug": inst.get("debug", 0),
                            "engine": inst["engine"],
                            "ins": [], "outs": [],
                            "name": inst["name"] + f"-lw{k}",
                            "opcode": "Drain",
                            "sync_info": {"on_update": [], "on_wait": [wk]},
                        })
                    si["on_wait"] = w[-1:]
                    changed = True
                out.append(inst)
            blk["instructions"] = out
    return orjson.dumps(m) if changed else bir_json


def _compile_bir_legalized(bir_json, tmpdir, neff_name="file.neff"):
    return _orig_compile_bir(_legalize_waits(bir_json), tmpdir, neff_name)


_bu.compile_bir_kernel = _compile_bir_legalized
_b2j.compile_bir_kernel = _compile_bir_legalized

F32 = mybir.dt.float32
F32R = mybir.dt.float32r
F16 = mybir.dt.float16
U32 = mybir.dt.uint32

B, S, H, E = 4, 4096, 4096, 64
N_CORES = 8
P = 128                      # partitions / tile height
TOK_TOTAL = B * S            # 16384
TOK = TOK_TOTAL // N_CORES   # 2048 tokens per core
NCH = H // P                 # 32 contraction chunks of 128
GTOK = 512                   # tokens per GEMM group (PSUM bank = 512 fp32)
NTPG = GTOK // P             # 4 x-tiles per group
NGRP = TOK // GTOK           # 4 groups per core
GEMM_LAG = 4                 # chunks the GEMM trails the transposes by

GEMM_MODE = "f32r"           # "f32r" | "fp16x3"


def build_nc(tok: int = TOK):
    """Build the per-core Bass program (SPMD: same program, 8 cores)."""
    nc = bass.Bass()

    # In f32r mode the whole x path is *declared* float32r so the BIR
    # verifier accepts the f32r GEMM; the bits are plain fp32 end-to-end
    # (DMA and transpose-mode PE moves don't round).
    XDT = F32R if GEMM_MODE == "f32r" else F32

    x_ext = nc.declare_dram_parameter("x", [tok, H], XDT, isOutput=False)
    id_ext = nc.declare_dram_parameter("ident", [P, P], XDT, isOutput=False)
    id2_ext = nc.declare_dram_parameter("ident2", [P, P], F32, isOutput=False)
    ow_ext = nc.declare_dram_parameter("out_w", [tok, 2], F32, isOutput=True)
    oi_ext = nc.declare_dram_parameter("out_i", [tok, 2], U32, isOutput=True)
    if GEMM_MODE == "f32r":
        wt_ext = nc.declare_dram_parameter("wt", [P, NCH, E], F32R,
                                           isOutput=False)
    else:
        whi_ext = nc.declare_dram_parameter("whi", [P, NCH, E], F16,
                                            isOutput=False)
        wlo_ext = nc.declare_dram_parameter("wlo", [P, NCH, E], F16,
                                            isOutput=False)

    with tile.TileContext(nc) as tc:
        with (
            tc.tile_pool(name="consts", bufs=1) as consts,
            tc.tile_pool(name="xin", bufs=9) as xpool,
            tc.tile_pool(name="xt", bufs=8) as xtpool,
            tc.tile_pool(name="small", bufs=4) as small,
            tc.tile_pool(name="outp", bufs=8) as outp,
        ):
            if GEMM_MODE == "f32r":
                wt_sb = consts.tile([P, NCH, E], F32R)
                nc.sync.dma_start(wt_sb[:], wt_ext[:])
            else:
                whi_sb = consts.tile([P, NCH, E], F16)
                nc.sync.dma_start(whi_sb[:], whi_ext[:])
                wlo_sb = consts.tile([P, NCH, E], F16)
                nc.sync.dma_start(wlo_sb[:], wlo_ext[:])
            id_sb = consts.tile([P, P], XDT)
            nc.sync.dma_start(id_sb[:], id_ext[:])
            id2_sb = consts.tile([P, P], F32)
            nc.sync.dma_start(id2_sb[:], id2_ext[:])

            # Primers: each engine's first instruction carries a preamble
            # self-guard wait and fused LDWEIGHTS can hold just one wait,
            # so give every engine a first op with no other dependency
            # (const APs are pre-TileContext, untracked), and absorb each
            # const-DMA sem into a throwaway PE op. The scr pool is opened
            # and closed before the main PSUM pools so its banks are free
            # for the pipeline.
            prim = consts.tile([P, 2], F32)
            nc.vector.memset(prim[:, 0:1], 0.0)
            nc.scalar.copy(prim[:, 1:2], nc.const_aps.tensor(1.0, (P, 1)))
            with tc.tile_pool(name="scr", bufs=1, space="PSUM") as scr_pool:
                scr = scr_pool.tile([P, P], XDT)
                nc.tensor.matmul(scr[:], id_sb[:], id_sb[:],
                                 is_transpose=True, start=True, stop=True)
                scr2 = scr_pool.tile([P, P], F32)
                nc.tensor.matmul(scr2[:], id2_sb[:], id2_sb[:],
                                 is_transpose=True, start=True, stop=True)
                if GEMM_MODE == "f32r":
                    nc.tensor.matmul(scr2[0:E, :], wt_sb[:, 0, :],
                                     id_sb[:],
                                     start=True, stop=True)
                else:
                    nc.tensor.matmul(scr[0:E, 0:E], whi_sb[:, 0, :],
                                     whi_sb[:, 0, :], start=True, stop=True)
                    nc.tensor.matmul(scr[0:E, 0:E], wlo_sb[:, 0, :],
                                     wlo_sb[:, 0, :], start=False, stop=True)

            def emit_gemm(lg_ps, xt_tiles, c):
                """logitsT[e, tok] += wT[c].T-free GEMM over chunk c."""
                if GEMM_MODE == "f32r":
                    nc.tensor.matmul(
                        lg_ps[:], wt_sb[:, c, :], xt_tiles[c][:],
                        start=(c == 0), stop=(c == NCH - 1),
                    )
                else:
                    xr = xt_tiles[c]
                    nc.tensor.matmul(lg_ps[:], whi_sb[:, c, :], xr["hi"][:],
                                     start=(c == 0), stop=False)
                    nc.tensor.matmul(lg_ps[:], wlo_sb[:, c, :], xr["hi"][:],
                                     start=False, stop=False)
                    nc.tensor.matmul(lg_ps[:], whi_sb[:, c, :], xr["lo"][:],
                                     start=False, stop=(c == NCH - 1))

            with (
                tc.tile_pool(name="ps_t", bufs=5, space="PSUM") as ps_t,
                tc.tile_pool(name="ps_l", bufs=2, space="PSUM") as ps_l,
                tc.tile_pool(name="ps_b", bufs=1, space="PSUM") as ps_b,
            ):
                copy_flip = 0

                def do_transpose(x_tiles, xt_tiles, c):
                    """Transpose chunk c of the group's tiles -> PSUM -> SBUF.

                    Tiles are processed in PAIRS with independent PSUM
                    accumulation groups, so a chunk's transposes only gate
                    on the pair's x-tile DMAs (not the whole group's)."""
                    nonlocal copy_flip
                    ntl = len(x_tiles)
                    gt = ntl * P
                    if GEMM_MODE == "f32r":
                        xT_sb = xtpool.tile([P, gt], XDT)
                    else:
                        hi_sb = xtpool.tile([P, gt], F16)
                        lo_sb = xtpool.tile([P, gt], F16)
                    for h0 in range(0, ntl, 2):
                        nh = min(2, ntl - h0)
                        xT_ps = ps_t.tile([P, 2, P], XDT)
                        for k in range(nh):
                            nc.tensor.matmul(
                                xT_ps[:, k, :],
                                x_tiles[h0 + k][:, c * P:(c + 1) * P],
                                id_sb[:],
                                is_transpose=True,
                                start=(k == 0),
                                stop=(k == nh - 1),
                            )
                        src = xT_ps[:, 0:nh, :]
                        if GEMM_MODE == "f32r":
                            dst = xT_sb[:, h0 * P:(h0 + nh) * P]
                            if copy_flip % 2 == 0:
                                nc.vector.tensor_copy(dst, src)
                            else:
                                nc.scalar.copy(dst, src)
                        else:
                            dh = hi_sb[:, h0 * P:(h0 + nh) * P]
                            dl = lo_sb[:, h0 * P:(h0 + nh) * P]
                            if copy_flip % 2 == 0:
                                nc.vector.tensor_copy(dh, src)
                                nc.scalar.tensor_tensor(
                                    dl, src, dh, mybir.AluOpType.subtract)
                            else:
                                nc.scalar.copy(dh, src)
                                nc.vector.tensor_tensor(
                                    dl, src, dh, mybir.AluOpType.subtract)
                        copy_flip += 1
                    if GEMM_MODE == "f32r":
                        xt_tiles[c] = xT_sb
                    else:
                        xt_tiles[c] = {"hi": hi_sb, "lo": lo_sb}

                def emit_output(tok0, ntl, lg_ps):
                    """Transpose logitsT back to [tok, e]; top-2 + DMA."""
                    gt = ntl * P
                    lgT_sb = small.tile([E, gt], F32)
                    nc.vector.tensor_copy(lgT_sb[:], lg_ps[:])
                    lg_bk = ps_b.tile([P, ntl, E], F32)
                    for j in range(ntl):
                        nc.tensor.matmul(
                            lg_bk[:, j, :],
                            lgT_sb[:, j * P:(j + 1) * P],
                            id2_sb[0:E, 0:E],
                            is_transpose=True,
                            start=(j == 0),
                            stop=(j == ntl - 1),
                        )
                    lg_sb = small.tile([P, ntl, E], F32)
                    nc.vector.tensor_copy(lg_sb[:], lg_bk[:])

                    for j in range(ntl):
                        mx = outp.tile([P, 8], F32)
                        nc.vector.max(mx[:], lg_sb[:, j, :])
                        ix = outp.tile([P, 8], U32)
                        nc.vector.max_index(ix[:], mx[:], lg_sb[:, j, :])

                        ow_t = outp.tile([P, 2], F32)
                        oi_t = outp.tile([P, 2], U32)
                        # w1 = sigmoid(l1 - l2); w2 symmetric
                        nc.scalar.activation(
                            ow_t[:, 0:1], mx[:, 1:2],
                            mybir.ActivationFunctionType.Sigmoid,
                            bias=mx[:, 0:1], scale=-1.0,
                        )
                        nc.scalar.activation(
                            ow_t[:, 1:2], mx[:, 0:1],
                            mybir.ActivationFunctionType.Sigmoid,
                            bias=mx[:, 1:2], scale=-1.0,
                        )
                        nc.vector.tensor_copy(oi_t[:], ix[:, 0:2])

                        t0 = tok0 + j * P
                        nc.sync.dma_start(ow_ext[t0:t0 + P, :], ow_t[:])
                        nc.sync.dma_start(oi_ext[t0:t0 + P, :], oi_t[:])

                groups = [(t, NTPG) for t in range(0, TOK, GTOK)]

                pending_out = None
                for (tok0, ntl) in groups:
                    x_tiles = []
                    for ti in range(ntl):
                        x_sb = xpool.tile([P, H], XDT)
                        t0 = tok0 + ti * P
                        nc.sync.dma_start(x_sb[:], x_ext[t0:t0 + P, :])
                        x_tiles.append(x_sb)

                    lg_ps = ps_l.tile([E, ntl * P], F32)
                    xt_tiles = {}

                    # software pipeline: transposes run GEMM_LAG chunks
                    # ahead; the previous group's output stage is emitted
                    # into this group's transpose stream.
                    for c in range(NCH):
                        do_transpose(x_tiles, xt_tiles, c)
                        if c == 3 and pending_out is not None:
                            emit_output(*pending_out)
                            pending_out = None
                        if c >= GEMM_LAG:
                            emit_gemm(lg_ps, xt_tiles, c - GEMM_LAG)
                            xt_tiles.pop(c - GEMM_LAG)
                    for c in range(NCH - GEMM_LAG, NCH):
                        emit_gemm(lg_ps, xt_tiles, c)
                        xt_tiles.pop(c)
                    pending_out = (tok0, ntl, lg_ps)

                emit_output(*pending_out)

    return nc


_NC_CACHE = {}


def _get_nc(tok: int):
    if tok not in _NC_CACHE:
        _NC_CACHE[tok] = build_nc(tok)
    return _NC_CACHE[tok]


def make_in_maps(x: np.ndarray, gate_w: np.ndarray):
    """Shard full inputs into per-core input maps."""
    xf = np.ascontiguousarray(x.reshape(TOK_TOTAL, H), dtype=np.float32)
    # wt[p, c, e] = gate_w[e, 128*c + p]
    wt = np.ascontiguousarray(
        gate_w.T.reshape(NCH, P, E).transpose(1, 0, 2), dtype=np.float32
    )
    ident = np.eye(P, dtype=np.float32)
    common = {"ident": ident, "ident2": ident}
    if GEMM_MODE == "f32r":
        common["wt"] = wt
    else:
        whi = wt.astype(np.float16)
        wlo = (wt - whi.astype(np.float32)).astype(np.float16)
        common["whi"] = whi
        common["wlo"] = wlo
    return [
        {"x": np.ascontiguousarray(xf[i * TOK:(i + 1) * TOK]), **common}
        for i in range(N_CORES)
    ]


def kernel(x, gate_w, _trace: bool = False):
    x = np.asarray(x, dtype=np.float32)
    gate_w = np.asarray(gate_w, dtype=np.float32)
    nc = _get_nc(TOK)
    in_maps = make_in_maps(x, gate_w)
    res = run_bass_kernel_spmd(
        nc, in_maps, core_ids=list(range(N_CORES)), trace=_trace
    )
    out_w = np.concatenate([res.results[i]["out_w"] for i in range(N_CORES)])
    out_i = np.concatenate([res.results[i]["out_i"] for i in range(N_CORES)])
    topk_weights = out_w.reshape(B, S, 2)
    topk_indices = out_i.astype(np.int32).reshape(B, S, 2)
    if _trace:
        kernel._last_result = res
    return topk_weights, topk_indices


# revision 31
# speedup vs baseline: 1.6363x; 1.6363x over previous
"""MoE router kernel for Trainium2 (8 NeuronCores, SPMD data-parallel).

Computes, for x [B,S,H] and gate_w [E,H]:
    logits = x @ gate_w.T           # [B,S,E]
    p = softmax(logits, -1)
    w, i = top_k(p, 2); w = w / w.sum(-1, keepdims=True)

Math used on-device: renormalized top-2 softmax weights collapse to
    w1 = sigmoid(l1 - l2), w2 = sigmoid(l2 - l1)
where l1 >= l2 are the top-2 logits, so the full softmax is never needed.

Sharding: tokens (B*S = 16384) split evenly across 8 cores; gate weights
replicated. Per core: 2048 tokens x 4096 hidden.

Per-core pipeline (groups of 512 tokens = 4 x-tiles of [128, 4096]):
  DMA x tile [128, 4096] fp32 -> PE fp32 transposes of 128x128 chunks
  into PSUM banks [128h, 512t] -> DVE/ACT copy to SBUF -> fp32r GEMM
  (w chunk [128h, 64e] stationary, xT [128h, 512t] moving, 1 cyc/row)
  accumulating logitsT [64e, 512t] in PSUM -> copy to SBUF -> 4 PE
  transposes back to [128t, 64e] -> DVE max/max_index (top-8 sorted) ->
  ACT sigmoid -> DMA out.

fp32r (reduced-precision fp32 matmul, ~12.7 effective mantissa bits
measured on HW) is used ONLY for the final contraction against the tiny
gate weights; x itself is transposed at full fp32. Measured index
mismatch stays well inside the 2e-2 gate. Set GEMM_MODE="fp16x3" for a
bit-near-fp32 3-pass split-fp16 GEMM instead (slower, ~20 mantissa
bits).
"""

import sys

sys.path.insert(0, "/opt/trn_rl_repo")

import numpy as np

import concourse.bass as bass
import concourse.mybir as mybir
import concourse.tile as tile
from concourse.bass_utils import run_bass_kernel_spmd
import orjson
import concourse.bass_utils as _bu
import concourse.bass2jax as _b2j

_orig_compile_bir = _bu.compile_bir_kernel


def _legalize_waits(bir_json: bytes) -> bytes:
    """This walrus build allows only ONE sync-wait per compute
    instruction; move excess waits onto a Drain inserted just before
    (Drain accepts one wait each)."""
    m = orjson.loads(bir_json)
    changed = False
    for fn in m["functions"]:
        for blk in fn["blocks"]:
            out = []
            for inst in blk["instructions"]:
                si = inst.get("sync_info")
                w = (si or {}).get("on_wait") or []
                if len(w) > 1:
                    for k, wk in enumerate(w[:-1]):
                        out.append({
                            "debug": inst.get("debug", 0),
                            "engine": inst["engine"],
                            "ins": [], "outs": [],
                            "name": inst["name"] + f"-lw{k}",
                            "opcode": "Drain",
                            "sync_info": {"on_update": [], "on_wait": [wk]},
                        })
                    si["on_wait"] = w[-1:]
                    changed = True
                out.append(inst)
            blk["instructions"] = out
    return orjson.dumps(m) if changed else bir_json


def _compile_bir_legalized(bir_json, tmpdir, neff_name="file.neff"):
    return _orig_compile_bir(_legalize_waits(bir_json), tmpdir, neff_name)


_bu.compile_bir_kernel = _compile_bir_legalized
_b2j.compile_bir_kernel = _compile_bir_legalized

F32 = mybir.dt.float32
F32R = mybir.dt.float32r
F16 = mybir.dt.float16
U32 = mybir.dt.uint32

B, S, H, E = 4, 4096, 4096, 64
N_CORES = 8
P = 128                      # partitions / tile height
TOK_TOTAL = B * S            # 16384
TOK = TOK_TOTAL // N_CORES   # 2048 tokens per core
NCH = H // P                 # 32 contraction chunks of 128
GTOK = 512                   # tokens per GEMM group (PSUM bank = 512 fp32)
NTPG = GTOK // P             # 4 x-tiles per group
NGRP = TOK // GTOK           # 4 groups per core
GEMM_LAG = 4                 # chunks the GEMM trails the transposes by

GEMM_MODE = "f32r"           # "f32r" | "fp16x3"


def build_nc(tok: int = TOK):
    """Build the per-core Bass program (SPMD: same program, 8 cores)."""
    nc = bass.Bass()

    # In f32r mode the whole x path is *declared* float32r so the BIR
    # verifier accepts the f32r GEMM; the bits are plain fp32 end-to-end
    # (DMA and transpose-mode PE moves don't round).
    XDT = F32R if GEMM_MODE == "f32r" else F32

    x_ext = nc.declare_dram_parameter("x", [tok, H], XDT, isOutput=False)
    id_ext = nc.declare_dram_parameter("ident", [P, P], XDT, isOutput=False)
    id2_ext = nc.declare_dram_parameter("ident2", [P, P], F32, isOutput=False)
    ow_ext = nc.declare_dram_parameter("out_w", [tok, 2], F32, isOutput=True)
    oi_ext = nc.declare_dram_parameter("out_i", [tok, 2], U32, isOutput=True)
    if GEMM_MODE == "f32r":
        wt_ext = nc.declare_dram_parameter("wt", [P, NCH, E], F32R,
                                           isOutput=False)
    else:
        whi_ext = nc.declare_dram_parameter("whi", [P, NCH, E], F16,
                                            isOutput=False)
        wlo_ext = nc.declare_dram_parameter("wlo", [P, NCH, E], F16,
                                            isOutput=False)

    with tile.TileContext(nc) as tc:
        with (
            tc.tile_pool(name="consts", bufs=1) as consts,
            tc.tile_pool(name="xin", bufs=9) as xpool,
            tc.tile_pool(name="xt", bufs=8) as xtpool,
            tc.tile_pool(name="small", bufs=4) as small,
            tc.tile_pool(name="outp", bufs=8) as outp,
        ):
            # Group 0's x tiles are DMA'd FIRST (on the sync queue) so the
            # PE can start transposing as early as possible; the small
            # consts go on the scalar HWDGE queue so they don't serialize
            # behind 8 MB of x on the sync queue.
            g0_tiles = []
            for ti in range(NTPG):
                x_sb = xpool.tile([P, H], XDT)
                nc.sync.dma_start(x_sb[:], x_ext[ti * P:(ti + 1) * P, :])
                g0_tiles.append(x_sb)

            if GEMM_MODE == "f32r":
                wt_sb = consts.tile([P, NCH, E], F32R)
                nc.scalar.dma_start(wt_sb[:], wt_ext[:])
            else:
                whi_sb = consts.tile([P, NCH, E], F16)
                nc.scalar.dma_start(whi_sb[:], whi_ext[:])
                wlo_sb = consts.tile([P, NCH, E], F16)
                nc.scalar.dma_start(wlo_sb[:], wlo_ext[:])
            id_sb = consts.tile([P, P], XDT)
            nc.scalar.dma_start(id_sb[:], id_ext[:])
            id2_sb = consts.tile([P, P], F32)
            nc.scalar.dma_start(id2_sb[:], id2_ext[:])

            # Primers: each engine's first instruction carries a preamble
            # self-guard wait and fused LDWEIGHTS can hold just one wait,
            # so give every engine a first op with no other dependency
            # (const APs are pre-TileContext, untracked), and absorb each
            # const-DMA sem into a throwaway PE op. The scr pool is opened
            # and closed before the main PSUM pools so its banks are free
            # for the pipeline.
            prim = consts.tile([P, 2], F32)
            nc.vector.memset(prim[:, 0:1], 0.0)
            nc.scalar.copy(prim[:, 1:2], nc.const_aps.tensor(1.0, (P, 1)))
            with tc.tile_pool(name="scr", bufs=1, space="PSUM") as scr_pool:
                scr = scr_pool.tile([P, P], XDT)
                nc.tensor.matmul(scr[:], id_sb[:], id_sb[:],
                                 is_transpose=True, start=True, stop=True)
                scr2 = scr_pool.tile([P, P], F32)
                nc.tensor.matmul(scr2[:], id2_sb[:], id2_sb[:],
                                 is_transpose=True, start=True, stop=True)
                if GEMM_MODE == "f32r":
                    nc.tensor.matmul(scr2[0:E, :], wt_sb[:, 0, :],
                                     id_sb[:],
                                     start=True, stop=True)
                else:
                    nc.tensor.matmul(scr[0:E, 0:E], whi_sb[:, 0, :],
                                     whi_sb[:, 0, :], start=True, stop=True)
                    nc.tensor.matmul(scr[0:E, 0:E], wlo_sb[:, 0, :],
                                     wlo_sb[:, 0, :], start=False, stop=True)

            def emit_gemm(lg_ps, xt_tiles, c):
                """logitsT[e, tok] += wT[c].T-free GEMM over chunk c."""
                if GEMM_MODE == "f32r":
                    nc.tensor.matmul(
                        lg_ps[:], wt_sb[:, c, :], xt_tiles[c][:],
                        start=(c == 0), stop=(c == NCH - 1),
                    )
                else:
                    xr = xt_tiles[c]
                    nc.tensor.matmul(lg_ps[:], whi_sb[:, c, :], xr["hi"][:],
                                     start=(c == 0), stop=False)
                    nc.tensor.matmul(lg_ps[:], wlo_sb[:, c, :], xr["hi"][:],
                                     start=False, stop=False)
                    nc.tensor.matmul(lg_ps[:], whi_sb[:, c, :], xr["lo"][:],
                                     start=False, stop=(c == NCH - 1))

            with (
                tc.tile_pool(name="ps_t", bufs=5, space="PSUM") as ps_t,
                tc.tile_pool(name="ps_l", bufs=2, space="PSUM") as ps_l,
                tc.tile_pool(name="ps_b", bufs=1, space="PSUM") as ps_b,
            ):
                copy_flip = 0

                def do_transpose(x_tiles, xt_tiles, c):
                    """Transpose chunk c of the group's tiles -> PSUM -> SBUF.

                    Tiles are processed in PAIRS with independent PSUM
                    accumulation groups, so a chunk's transposes only gate
                    on the pair's x-tile DMAs (not the whole group's)."""
                    nonlocal copy_flip
                    ntl = len(x_tiles)
                    gt = ntl * P
                    xT_ps = ps_t.tile([P, ntl, P], XDT)
                    for ti in range(ntl):
                        nc.tensor.matmul(
                            xT_ps[:, ti, :],
                            x_tiles[ti][:, c * P:(c + 1) * P],
                            id_sb[:],
                            is_transpose=True,
                            start=(ti == 0),
                            stop=(ti == ntl - 1),
                        )
                    if GEMM_MODE == "f32r":
                        xT_sb = xtpool.tile([P, gt], XDT)
                        if copy_flip % 2 == 0:
                            nc.vector.tensor_copy(xT_sb[:], xT_ps[:])
                        else:
                            nc.scalar.copy(xT_sb[:], xT_ps[:])
                        copy_flip += 1
                        xt_tiles[c] = xT_sb
                    else:
                        hi = xtpool.tile([P, gt], F16)
                        lo = xtpool.tile([P, gt], F16)
                        if copy_flip % 2 == 0:
                            nc.vector.tensor_copy(hi[:], xT_ps[:])
                            nc.scalar.tensor_tensor(
                                lo[:], xT_ps[:], hi[:],
                                mybir.AluOpType.subtract)
                        else:
                            nc.scalar.copy(hi[:], xT_ps[:])
                            nc.vector.tensor_tensor(
                                lo[:], xT_ps[:], hi[:],
                                mybir.AluOpType.subtract)
                        copy_flip += 1
                        xt_tiles[c] = {"hi": hi, "lo": lo}

                def emit_output(tok0, ntl, lg_ps):
                    """Transpose logitsT back to [tok, e]; top-2 + DMA."""
                    gt = ntl * P
                    lgT_sb = small.tile([E, gt], F32)
                    nc.vector.tensor_copy(lgT_sb[:], lg_ps[:])
                    lg_bk = ps_b.tile([P, ntl, E], F32)
                    for j in range(ntl):
                        nc.tensor.matmul(
                            lg_bk[:, j, :],
                            lgT_sb[:, j * P:(j + 1) * P],
                            id2_sb[0:E, 0:E],
                            is_transpose=True,
                            start=(j == 0),
                            stop=(j == ntl - 1),
                        )
                    lg_sb = small.tile([P, ntl, E], F32)
                    nc.vector.tensor_copy(lg_sb[:], lg_bk[:])

                    for j in range(ntl):
                        mx = outp.tile([P, 8], F32)
                        nc.vector.max(mx[:], lg_sb[:, j, :])
                        ix = outp.tile([P, 8], U32)
                        nc.vector.max_index(ix[:], mx[:], lg_sb[:, j, :])

                        # ship the top-2 logits and indices; the sigmoid
                        # renormalization runs on the host (tiny)
                        t0 = tok0 + j * P
                        nc.sync.dma_start(ow_ext[t0:t0 + P, :], mx[:, 0:2])
                        nc.sync.dma_start(oi_ext[t0:t0 + P, :], ix[:, 0:2])

                groups = [(t, NTPG) for t in range(0, TOK, GTOK)]

                pending_out = None
                for (tok0, ntl) in groups:
                    if tok0 == 0:
                        x_tiles = g0_tiles
                    else:
                        x_tiles = []
                        for ti in range(ntl):
                            x_sb = xpool.tile([P, H], XDT)
                            t0 = tok0 + ti * P
                            nc.sync.dma_start(x_sb[:], x_ext[t0:t0 + P, :])
                            x_tiles.append(x_sb)

                    lg_ps = ps_l.tile([E, ntl * P], F32)
                    xt_tiles = {}

                    # software pipeline: transposes run GEMM_LAG chunks
                    # ahead; the previous group's output stage is emitted
                    # into this group's transpose stream.
                    for c in range(NCH):
                        do_transpose(x_tiles, xt_tiles, c)
                        if c == 3 and pending_out is not None:
                            emit_output(*pending_out)
                            pending_out = None
                        if c >= GEMM_LAG:
                            emit_gemm(lg_ps, xt_tiles, c - GEMM_LAG)
                            xt_tiles.pop(c - GEMM_LAG)
                    for c in range(NCH - GEMM_LAG, NCH):
                        emit_gemm(lg_ps, xt_tiles, c)
                        xt_tiles.pop(c)
                    pending_out = (tok0, ntl, lg_ps)

                emit_output(*pending_out)

    return nc


_NC_CACHE = {}


def _get_nc(tok: int):
    if tok not in _NC_CACHE:
        _NC_CACHE[tok] = build_nc(tok)
    return _NC_CACHE[tok]


def make_in_maps(x: np.ndarray, gate_w: np.ndarray):
    """Shard full inputs into per-core input maps."""
    xf = np.ascontiguousarray(x.reshape(TOK_TOTAL, H), dtype=np.float32)
    # wt[p, c, e] = gate_w[e, 128*c + p]
    wt = np.ascontiguousarray(
        gate_w.T.reshape(NCH, P, E).transpose(1, 0, 2), dtype=np.float32
    )
    ident = np.eye(P, dtype=np.float32)
    common = {"ident": ident, "ident2": ident}
    if GEMM_MODE == "f32r":
        common["wt"] = wt
    else:
        whi = wt.astype(np.float16)
        wlo = (wt - whi.astype(np.float32)).astype(np.float16)
        common["whi"] = whi
        common["wlo"] = wlo
    return [
        {"x": np.ascontiguousarray(xf[i * TOK:(i + 1) * TOK]), **common}
        for i in range(N_CORES)
    ]


def kernel(x, gate_w, _trace: bool = False):
    x = np.asarray(x, dtype=np.float32)
    gate_w = np.asarray(gate_w, dtype=np.float32)
    nc = _get_nc(TOK)
    in_maps = make_in_maps(x, gate_w)
    res = run_bass_kernel_spmd(
        nc, in_maps, core_ids=list(range(N_CORES)), trace=_trace
    )
    lg12 = np.concatenate([res.results[i]["out_w"] for i in range(N_CORES)])
    out_i = np.concatenate([res.results[i]["out_i"] for i in range(N_CORES)])
    # device ships the top-2 logits (l1, l2); renormalized top-2 softmax
    # weights collapse to sigmoids of the logit difference
    d12 = lg12[:, 0] - lg12[:, 1]
    w1 = 1.0 / (1.0 + np.exp(-d12))
    w2 = 1.0 / (1.0 + np.exp(d12))
    topk_weights = np.stack([w1, w2], axis=-1).astype(np.float32).reshape(B, S, 2)
    topk_indices = out_i.astype(np.int32).reshape(B, S, 2)
    if _trace:
        kernel._last_result = res
    return topk_weights, topk_indices
